# revision 3
# baseline (speedup 1.0000x reference)
"""Trainium2 Bass kernel for MultiHeadAttentionRoPE — bf16 optimized.

Problem (hardcoded): B=2, S=2048, D=1024, H=16 heads, Dh=64, fp32 in/out.
    qkv = x @ w_qkv ; q,k -> RoPE ; causal attention ; out = ctx @ w_proj

Sharding: tensor-parallel over heads across 8 cores (2 heads/core).
Each core reads the full x (transposed, bf16), its slice of w_qkv/w_proj,
computes attention for its 2 heads and a *partial* projection output in
bf16; the host sums the 8 partials in fp32 (the gather step of
row-parallel TP, replacing the all-reduce).

Key optimizations over the fp32 baseline:
  - all SBUF data and matmul operands in bf16 (PSUM accumulates fp32):
    halves DMA bytes, 2-4x DVE throughput, full-rate PE.
  - RoPE rotate-half as a DVE stream_shuffle: the q/k feature pairs
    (f, f+32) are interleaved onto adjacent partitions by a host-side
    permutation of the wq/wk columns (+ matching cos/sin tables), making
    the rotation a within-quadrant lane swap. No partition-swap DMAs.
  - one DMA per x chunk / per half output qb block (batched APs).
  - ACT runs only the softmax exp; PSUM drains go to DVE/Pool.
  - single shared 6-bank PSUM ring for qkv/scores/pbc/proj tiles; score
    matmuls run one kc-block ahead of the PV accumulation (software
    pipeline) so the PE never waits on the exp.
  - stage1 of batch 1 is interleaved into attention of batch 0 at
    kc-block granularity to fill ACT-bound stalls with PE work.
"""

import functools
import os
import sys

import numpy as np

sys.path.insert(0, "/opt/trn_rl_repo")

# ---- problem constants (must match reference.py) ----
B = 2
S = 2048
D = 1024
H = 16
Dh = 64
N_CORES = 8
HPC = H // N_CORES          # heads per core = 2
KC = D // 128               # contraction chunks = 8
TCH = 512                   # token chunk for stage 1
NTCH = S // TCH             # 4 chunks per batch
NSUB = S // 128             # 16 key subchunks per batch
ROPE_BASE = 10000.0
SCALE = 1.0 / 8.0           # 1/sqrt(Dh)

SWAP_MASK = [i ^ 1 for i in range(32)]   # adjacent-lane swap per quadrant


def _build_program(loop_n=1, phases="all", opts=""):
    import concourse.bass as bass
    opts = set(opts.split(",")) if opts else set()
    import concourse.mybir as mybir
    import concourse.tile as tile
    from concourse import bacc
    from contextlib import ExitStack

    FP = mybir.dt.float32
    FPR = mybir.dt.float32r
    BF = mybir.dt.bfloat16
    EXP = mybir.ActivationFunctionType.Exp
    R = lambda ap: ap.bitcast(FPR)

    nc = bacc.Bacc("TRN2", target_bir_lowering=False, debug=False)

    # host supplies p-major layouts so every DMA is a single dense AP
    xt_d = nc.dram_tensor("xt", [B, 128, KC, S], BF, kind="ExternalInput").ap()
    wqk_d = nc.dram_tensor("wqk", [128, KC, 3 * 128], BF, kind="ExternalInput").ap()
    wproj_d = nc.dram_tensor("wproj", [128, D], BF, kind="ExternalInput").ap()
    cos_d = nc.dram_tensor("cost", [128, S], BF, kind="ExternalInput").ap()
    sin_d = nc.dram_tensor("sint", [128, S], BF, kind="ExternalInput").ap()
    tri_d = nc.dram_tensor("tri", [128, 128], BF, kind="ExternalInput").ap()
    ident_d = nc.dram_tensor("ident", [128, 128], BF, kind="ExternalInput").ap()
    ones65_d = nc.dram_tensor("ones65", [65, 64], BF, kind="ExternalInput").ap()
    out_d = nc.dram_tensor("out", [B, S, D], BF, kind="ExternalOutput").ap()

    with tile.TileContext(nc) as tc, ExitStack() as ctx:
        consts = ctx.enter_context(tc.tile_pool(name="consts", bufs=1))
        store = ctx.enter_context(tc.tile_pool(name="store", bufs=1))
        xt_pool = ctx.enter_context(tc.tile_pool(name="xt_pool", bufs=4))
        rope_pool = ctx.enter_context(tc.tile_pool(name="rope_pool", bufs=2))
        p_pool = ctx.enter_context(tc.tile_pool(name="p_pool", bufs=4))
        pvc_pool = ctx.enter_context(tc.tile_pool(name="pvc_pool", bufs=4))
        nrm_pool = ctx.enter_context(tc.tile_pool(name="nrm_pool", bufs=3))
        ob_pool = ctx.enter_context(tc.tile_pool(name="ob_pool", bufs=2))
        ps_big = ctx.enter_context(tc.tile_pool(name="ps_big", bufs=2, space="PSUM"))
        ps_pv = ctx.enter_context(tc.tile_pool(name="ps_pv", bufs=2, space="PSUM"))
        ps_vo = ctx.enter_context(tc.tile_pool(name="ps_vo", bufs=2, space="PSUM"))

        # ---- constants ----
        wqk_sb = consts.tile([128, KC, 384], BF, name="wqk_sb")
        nc.sync.dma_start(wqk_sb[:, 0, :], wqk_d[:, 0, :])
        nc.sync.dma_start(wqk_sb[:, 1:, :], wqk_d[:, 1:, :])
        ident_sb = consts.tile([128, 128], BF, name="ident_sb")
        wproj_sb = consts.tile([128, D], BF, name="wproj_sb")
        cos_sb = consts.tile([128, S], BF, name="cos_sb")
        sin_sb = consts.tile([128, S], BF, name="sin_sb")
        tri_sb = consts.tile([128, 128], BF, name="tri_sb")
        ones65_sb = consts.tile([65, 64], BF, name="ones65_sb")

        def late_consts():
            nc.sync.dma_start(cos_sb, cos_d)
            nc.sync.dma_start(sin_sb, sin_d)
            nc.sync.dma_start(ident_sb, ident_d)
            nc.sync.dma_start(tri_sb, tri_d)
            nc.sync.dma_start(wproj_sb, wproj_d)
            nc.sync.dma_start(ones65_sb, ones65_d)
            for bb in range(B):
                nc.vector.memset(vt[bb][:, :, 64:65], 1.0)
                nc.vector.memset(vt[bb][:, :, 129:130], 1.0)

        # ---- persistent per-batch storage (all bf16) ----
        qT = {}   # b -> (128, S)  rows: [h0 | h1] interleaved rope pairs
        kT = {}
        vt = {}   # b -> (128, 16, 130) cols: [v_h0 | 1 | v_h1 | 1] per subchunk
        ctxT = {}
        for b in range(B):
            ctxT[b] = store.tile([128, S], BF, name=f"ctxT_{b}", tag=f"ctxT_{b}")
            qT[b] = store.tile([128, S], BF, name=f"qT_{b}", tag=f"qT_{b}")
            kT[b] = store.tile([128, S], BF, name=f"kT_{b}", tag=f"kT_{b}")
            vt[b] = store.tile([128, NSUB, 130], BF, name=f"vt_{b}", tag=f"vt_{b}")

        PCH = 2 * TCH           # tokens per stage-1 pair

        def _stage1_drain(b, tp, fc, pqk):
            psl = slice(tp * PCH, (tp + 1) * PCH)
            if fc < 2:
                dest = (qT[b] if fc == 0 else kT[b])[:, psl]
                tmp = rope_pool.tile([128, PCH], BF, name="rtmp", tag="rtmp")
                if "dvedrain" in opts:
                    nc.vector.tensor_copy(out=tmp, in_=pqk)
                else:
                    nc.scalar.copy(tmp, pqk)
                # rotate-half: adjacent-lane swap (pairs pre-interleaved on host)
                qs = rope_pool.tile([128, PCH], BF, name="rqs", tag="rqs")
                nc.vector.stream_shuffle(qs, tmp, SWAP_MASK)
                t1 = rope_pool.tile([128, PCH], BF, name="rt1", tag="rt1")
                nc.vector.tensor_mul(out=t1, in0=qs, in1=sin_sb[:, psl])
                nc.vector.tensor_mul(out=dest, in0=tmp, in1=cos_sb[:, psl])
                nc.vector.tensor_add(out=dest, in0=dest, in1=t1)
            else:
                # vT -> transpose to natural layout via PE (bf16, into
                # bitcast views of fp32 vo tiles)
                vts = rope_pool.tile([128, PCH], BF, name="vts", tag="vts")
                if "dvedrain" in opts:
                    nc.vector.tensor_copy(out=vts, in_=pqk)
                else:
                    nc.scalar.copy(vts, pqk)
                for half in range(2):
                    po = ps_vo.tile([128, 512], FP, name="po", tag="ps_vo")
                    pv4 = po.bitcast(BF)[:, 0:512].rearrange(
                        "p (s c) -> p s c", c=128)
                    for sc4 in range(4):
                        sc = half * 4 + sc4
                        nc.tensor.transpose(
                            pv4[:, sc4, :], vts[:, sc * 128:(sc + 1) * 128],
                            ident_sb,
                        )
                    s0 = tp * 8 + half * 4
                    v2 = vt[b][:, s0:s0 + 4, 0:130].rearrange(
                        "p s (a c) -> p s a c", c=65)[:, :, :, 0:64]
                    s2 = pv4.rearrange("p s (a c) -> p s a c", c=64)
                    nc.vector.tensor_copy(out=v2, in_=s2)

        def stage1_units(b, first=False):
            """Generator of emission units: both load units first (prefetch),
            then one unit per (pair, fc). The pair accumulates into one wide
            PSUM tile so weight loads serve two consecutive matmuls."""
            npair = 2
            xts = {}

            def load_unit(tp, split):
                def emit():
                    for i in range(npair):
                        t = npair * tp + i
                        xts[tp, i] = xt_pool.tile(
                            [128, KC, TCH], BF, name="xtile", tag="xt")
                    if split:
                        # first pair ever: half-chunk DMAs on the Pool queue
                        # (parallel to wqk on SP) so fc0 starts early
                        for kc0 in (0, 4):
                            for i in range(npair):
                                t = npair * tp + i
                                tsl = slice(t * TCH, (t + 1) * TCH)
                                nc.gpsimd.dma_start(
                                    xts[tp, i][:, kc0:kc0 + 4, :],
                                    xt_d[b, :, kc0:kc0 + 4, tsl])
                        late_consts()
                    else:
                        for i in range(npair):
                            t = npair * tp + i
                            tsl = slice(t * TCH, (t + 1) * TCH)
                            nc.sync.dma_start(xts[tp, i], xt_d[b, :, :, tsl])
                return emit

            def fc_unit(tp, fc):
                def emit():
                    psq = ps_big.tile([128, 2, TCH], FP, name="pqk", tag="big")
                    for kc in range(KC):
                        for i in range(npair):
                            nc.tensor.matmul(
                                psq[:, i, :],
                                lhsT=wqk_sb[:, kc, fc * 128:(fc + 1) * 128],
                                rhs=xts[tp, i][:, kc, :],
                                start=(kc == 0),
                                stop=(kc == KC - 1),
                            )
                    _stage1_drain(b, tp, fc, psq.rearrange("p i t -> p (i t)"))
                return emit

            for tp in range(NTCH // npair):
                yield load_unit(tp, split=(first and tp == 0))
            for tp in range(NTCH // npair):
                for fc in range(3):
                    yield fc_unit(tp, fc)

        def attention_qb(b, qb, tails=None, feed=None, feed_every=3):
            """Causal attention for one 512-query block; returns deferred
            tail (normalize + projection) closures. Score matmuls run one
            kc-block ahead of PV; pending tails / stage1 feeder units are
            injected between kc blocks to fill ACT-bound PE idle time."""
            qsl = slice(qb * TCH, (qb + 1) * TCH)
            nkc = 4 * qb + 4
            pv = ps_pv.tile([65, HPC, TCH], FP, name="ppv", tag="ps_pv", bufs=1)

            def scores(kc):
                off = max(0, (kc - 4 * qb) * 128)
                nv = TCH - off
                s2 = ps_big.tile([128, 2, TCH], FP, name="s2", tag="big")
                for h in range(HPC):
                    hb = h * 64
                    nc.tensor.matmul(
                        s2[:, h, :nv],
                        lhsT=kT[b][hb:hb + 64, kc * 128:(kc + 1) * 128],
                        rhs=qT[b][hb:hb + 64, qb * TCH + off:(qb + 1) * TCH],
                        start=True,
                        stop=True,
                    )
                p2 = p_pool.tile([128, 2, TCH], BF, name="p2", tag="p2")
                nc.scalar.activation(
                    p2[:, :, off:TCH], s2[:, :, 0:nv], EXP, scale=SCALE)
                if kc >= 4 * qb:  # diagonal band: triangular mask
                    if "mask1" not in opts:
                        import concourse.bass as _bass
                        t2 = _bass.AP(
                            tri_sb.tensor, tri_sb.offset,
                            [tri_sb.ap[0], [0, 2], [1, 128]])
                        nc.vector.tensor_mul(
                            out=p2[:, :, off:off + 128],
                            in0=p2[:, :, off:off + 128],
                            in1=t2,
                        )
                    else:
                        for h in range(HPC):
                            nc.vector.tensor_mul(
                                out=p2[:, h, off:off + 128],
                                in0=p2[:, h, off:off + 128],
                                in1=tri_sb,
                            )
                return p2

            def pv_acc(kc, p2):
                off = max(0, (kc - 4 * qb) * 128)
                for h in range(HPC):
                    nc.tensor.matmul(
                        pv[:, h, off:TCH],
                        lhsT=vt[b][:, kc, 65 * h:65 * h + 65],
                        rhs=p2[:, h, off:TCH],
                        start=(kc == 0),
                        stop=(kc == nkc - 1),
                    )

            prev = None
            since_feed = 0
            for kc in range(nkc):
                p2 = scores(kc)
                if prev is not None:
                    pv_acc(prev[0], prev[1])
                prev = (kc, p2)
                since_feed += 1
                if kc == 1 or since_feed >= feed_every:
                    if tails:
                        tails.pop(0)()
                        since_feed = 0
                    elif feed:
                        feed.pop(0)()
                        since_feed = 0
            pv_acc(prev[0], prev[1])

            # drain the accumulators to SBUF so the PSUM banks free up for
            # the next qb; the tail reads the SBUF copies.
            pvc = pvc_pool.tile([65, HPC, TCH], BF, name="pvc", tag="pvc", bufs=2)
            nc.vector.tensor_copy(out=pvc, in_=pv)
            return [lambda: _norm_tail(b, qb, qsl, pvc),
                    lambda: _proj_tail(b, qb, qsl)]

        def _norm_tail(b, qb, qsl, pvc):
            # normalize: 1/denom (row 64) broadcast across partitions with a
            # K=1 matmul per head.
            for h in range(HPC):
                hb = h * 64
                pbc = ps_vo.tile([128, 512], FP, name="po", tag="ps_vo")
                nc.tensor.matmul(
                    pbc[0:64, :], lhsT=ones65_sb[64:65, :], rhs=pvc[64:65, h, :],
                    start=True, stop=True,
                )
                rcb = nrm_pool.tile([64, TCH], FP, name="rcb", tag="rcb")
                nc.vector.reciprocal(rcb, pbc[0:64, :])
                if h == 0:
                    nc.vector.tensor_mul(
                        out=ctxT[b][0:64, qsl], in0=pvc[0:64, h, :], in1=rcb
                    )
                else:
                    ctmp = nrm_pool.tile([64, TCH], BF, name="ctmp", tag="ctmp")
                    nc.vector.tensor_mul(out=ctmp, in0=pvc[0:64, h, :], in1=rcb)
                    nc.gpsimd.dma_start(ctxT[b][hb:hb + 64, qsl], ctmp)

        def _proj_tail(b, qb, qsl):
            # fused projection for this qb's token range; one DMA per 2 tb
            ob = ob_pool.tile([128, 4, D], BF, name="ob", tag="ob")
            for tbl in range(4):
                tb = 4 * qb + tbl
                for ec in range(2):
                    po = ps_vo.tile([128, 512], FP, name="po", tag="ps_vo")
                    nc.tensor.matmul(
                        po,
                        lhsT=ctxT[b][:, tb * 128:(tb + 1) * 128],
                        rhs=wproj_sb[:, ec * 512:(ec + 1) * 512],
                        start=True,
                        stop=True,
                    )
                    dst = ob[:, tbl, ec * 512:(ec + 1) * 512]
                    if ec == 0:
                        nc.scalar.copy(dst, po)
                    else:
                        nc.vector.tensor_copy(out=dst, in_=po)
                if tbl % 2 == 1:
                    t0 = 4 * qb + tbl - 1
                    ov = out_d[b, t0 * 128:(t0 + 2) * 128, :].rearrange(
                        "(tb p) d -> p tb d", p=128)
                    nc.sync.dma_start(ov, ob[:, tbl - 1:tbl + 1, :])

        def whole():
            u0 = list(stage1_units(0, first=True))  # l0 l1 f00 f01 f02 f10 f11 f12
            if phases == "s1":
                for unit in u0:
                    unit()
                for unit in stage1_units(1):
                    unit()
                return
            if "inject" not in opts:
                for unit in u0:
                    unit()
                for q in range(NTCH):
                    for f in attention_qb(0, q):
                        f()
                for unit in stage1_units(1):
                    unit()
                for q in (3, 2, 1, 0):
                    for f in attention_qb(1, q):
                        f()
                return
            for unit in u0[:5]:     # loads + pair-0 fc units
                unit()
            tails = []
            feed = u0[5:] + list(stage1_units(1))
            tails.extend(attention_qb(0, 0, tails=tails, feed=feed))
            tails.extend(attention_qb(0, 1, tails=tails, feed=feed))
            while len(feed) > 8:    # pair-1 of batch 0 must precede qb 2/3
                feed.pop(0)()
            tails.extend(attention_qb(0, 2, tails=tails, feed=feed))
            tails.extend(attention_qb(0, 3, tails=tails, feed=feed))
            while feed:             # stage1(1) must fully precede attention(1)
                feed.pop(0)()
            for q in (3, 2, 1, 0):
                tails.extend(attention_qb(1, q, tails=tails, feed=feed))
            for unit in tails:
                unit()

        if loop_n == 1:
            whole()
        else:
            with tc.For_i(0, loop_n, 1):
                whole()

    nc.compile()
    return nc


@functools.lru_cache(maxsize=4)
def _get_program(loop_n=1, phases="all", opts=""):
    return _build_program(loop_n, phases, opts)


def _rope_tables():
    """Per-partition cos / sign-folded sin tables with the pair-interleave
    permutation: partition 2i <- feature i, partition 2i+1 <- feature 32+i
    (within each head's 64-row block)."""
    inv_freq = 1.0 / (ROPE_BASE ** (np.arange(0, Dh, 2, dtype=np.float32) / Dh))
    tpos = np.arange(S, dtype=np.float32)
    freqs = np.outer(tpos, inv_freq)                      # (S, 32)
    cosT = np.cos(freqs).T.astype(np.float32)             # (32, S) half-table
    sinT = np.sin(freqs).T.astype(np.float32)
    cos64 = np.empty((64, S), np.float32)
    sinf64 = np.empty((64, S), np.float32)
    cos64[0::2] = cosT
    cos64[1::2] = cosT
    sinf64[0::2] = -sinT
    sinf64[1::2] = sinT
    return np.tile(cos64, (2, 1)), np.tile(sinf64, (2, 1))   # (128, S)


def _perm64():
    """Column permutation applied to each head's wq/wk slice: new column j
    comes from old feature perm[j]."""
    p = np.empty(64, np.int64)
    p[0::2] = np.arange(32)
    p[1::2] = np.arange(32) + 32
    return p


def _host_inputs(x, w_qkv, w_proj):
    """Build the 8 per-core input maps from the full problem inputs."""
    import ml_dtypes
    bf16 = ml_dtypes.bfloat16

    x = np.asarray(x, dtype=np.float32)
    w_qkv = np.asarray(w_qkv, dtype=np.float32)
    w_proj = np.asarray(w_proj, dtype=np.float32)

    # (B, S, D) -> (B, D, S) -> (B, 128, KC, S): partition-major for 1-DMA loads
    xt = np.ascontiguousarray(
        x.transpose(0, 2, 1).reshape(B, KC, 128, S).transpose(0, 2, 1, 3)
    ).astype(bf16)

    cos_full, sin_full = _rope_tables()
    perm = _perm64()

    r = np.arange(128)
    tri = (r[None, :] >= r[:, None]).astype(np.float32)   # tri[r, c] = c >= r

    sel2 = np.zeros((2, 128), np.float32)
    sel2[0, 0:64] = 1.0
    sel2[1, 64:128] = 1.0

    wq = w_qkv[:, 0:D]
    wk = w_qkv[:, D:2 * D]
    wv = w_qkv[:, 2 * D:3 * D]

    in_maps = []
    for c in range(N_CORES):
        h0, h1 = 2 * c, 2 * c + 1
        cols = np.r_[h0 * 64:(h0 + 1) * 64, h1 * 64:(h1 + 1) * 64]
        # apply the rope-pair interleave within each head's 64 columns
        pcols = np.concatenate([cols[0:64][perm], cols[64:128][perm]])
        wqk_c = np.concatenate([wq[:, pcols], wk[:, pcols], wv[:, cols]], axis=1)
        # (D, 384) -> (KC, 128, 384) -> (128, KC, 384) partition-major
        wqk_c = np.ascontiguousarray(
            wqk_c.reshape(KC, 128, 384).transpose(1, 0, 2))
        in_maps.append({
            "xt": xt,
            "wqk": wqk_c.astype(bf16),
            "wproj": np.ascontiguousarray(
                w_proj[c * 128:(c + 1) * 128, :]).astype(bf16),
            "cost": cos_full.astype(bf16),
            "sint": sin_full.astype(bf16),
            "tri": tri.astype(bf16),
            "ident": np.eye(128, dtype=np.float32).astype(bf16),
            "ones65": np.ones((65, 64), dtype=np.float32).astype(bf16),
        })
    return in_maps


_last_results = None


def kernel(x, w_qkv, w_proj):
    global _last_results
    from concourse.bass_utils import run_bass_kernel_spmd

    nc = _get_program(1, "all", os.environ.get("KERNEL_OPTS", ""))
    in_maps = _host_inputs(x, w_qkv, w_proj)
    trace = bool(int(os.environ.get("KERNEL_TRACE", "0")))
    kwargs = {}
    if trace:
        kwargs["trace"] = True
        kwargs["trace_cores"] = list(range(N_CORES))
    res = run_bass_kernel_spmd(nc, in_maps, core_ids=list(range(N_CORES)), **kwargs)
    _last_results = res
    acc = np.zeros((B, S, D), dtype=np.float64)
    for r in res.results:
        acc += np.asarray(r["out"], dtype=np.float64)
    return acc.astype(np.float32)
